# revision 24
# baseline (speedup 1.0000x reference)
"""Eisner DP chart fill (nn_EsinerAgent) on 8 Trainium2 NeuronCores.

kernel(b_vinfo_mtx [64,128,128] f32, b_buffer_size [64] i32)
  -> (scores [64,128,128,2,2] f32, backtrace [64,128,128,2,2] i32)

Batch sharded 8 sentences/core (embarrassingly data-parallel); within a core
the chart fill is parallel over span-start i (partitions) and split q (free).

Layouts (per core, S=8 sentences), phase 1 (k <= 64):
  A/C/E/S00 : natural skewed charts [128(part=i), 8(s), 128(w)]:
              chart[i,s,w] = S_xx[i, i+w];  A=S11, C=S01, E=S10.
  Brev/Drev/Frev : sliding end-indexed charts [128(p), 128(slot), 8(s)]
              at step k: buf[k%3][p, m, s] = S_xx[p+m, p+k]
  Step k (j=i+k), m in [0,k-1] (q=i+m):
    base[i,m] = A[i,m] + Brev[i,m+1]   (S11[i,q] + S01[q+1,j])
    c01[i,m]  = C[i,m] + Drev[i,m]     (S01[i,q] + S00[q,j]),  m>=1
    c11[i,m]  = E[i,m] + Frev[i,m]     (S10[i,q] + S11[q,j]),  m>=1
  Slides via PE shift-permutation matmuls (fp32-exact).
Phase 2 (k >= 65): valid span starts are i < 128-k <= 63, so sentences 4-7
  are repacked onto partitions 64-127 (one-time shift-by-64 permutation
  matmuls at the k=64->65 transition); all per-step tensors shrink from
  [128, 8, k] to [128, 4, k], halving every big pass for the back half of
  the DP.  Cross-boundary slide pollution (partitions 62-63, 126-127) only
  touches invalid cells (i+k>127), which the host masks.
"""
import numpy as np
from contextlib import ExitStack

import concourse.bacc as bacc
import concourse.tile as tile
from concourse import mybir
from concourse.bass_types import AP
from concourse import bass_utils

N = 128
S = 8
NCORES = 8
DT = mybir.dt.float32
DI = mybir.dt.int32
NEGC = -9999.0
BON = 5.0
BIG = 128.0
KSPLIT = 64   # first step in the packed 4-lane layout (valid i < 64)
KSPLIT2 = 128  # first step in the packed 2-lane layout (128 = disabled)

IN_SPECS = {
    "vpc": [S, N, 2 * N],
    "vpcT": [S, N, 2 * N],
    "vl2": [N, 4, N],
    "vr2": [N, 4, N],
    "vl3": [N, 2, N],
    "vr3": [N, 2, N],
    "shf1": [N, N],
    "shf2": [N, N],
    "shf64": [N, N],
    "shf32": [N, N],
    "wmat": [N, N],
    "iota": [N, S],
    "iota2": [N, 4],
    "iota3": [N, 2],
}
OUT_NAMES = ["sc00", "sc01", "sc10", "sc11", "bt00", "bt01", "bt10", "bt11"]


def _host_consts():
    sh1 = np.zeros((N, N), np.float32)
    sh2 = np.zeros((N, N), np.float32)
    sh64 = np.zeros((N, N), np.float32)
    for p in range(N - 1):
        sh1[p + 1, p] = 1.0        # lhsT[r,p]=1 iff r=p+1 -> out[p]=in[p+1]
    for p in range(N - 2):
        sh2[p + 2, p] = 1.0
    for p in range(64, N):
        sh64[p - 64, p] = 1.0      # out[p] = in[p-64] for p >= 64
    sh32 = np.zeros((N, N), np.float32)
    for p in range(32, N):
        sh32[p - 32, p] = 1.0      # out[p] = in[p-32] for p >= 32
    w = np.broadcast_to((BIG - np.arange(N)).astype(np.float32)[None, :], (N, N)).copy()
    io = np.broadcast_to(np.arange(N, dtype=np.float32)[:, None], (N, S)).copy()
    io2 = np.broadcast_to((np.arange(N) % 64).astype(np.float32)[:, None],
                          (N, 4)).copy()
    io3 = np.broadcast_to((np.arange(N) % 32).astype(np.float32)[:, None],
                          (N, 2)).copy()
    return {"shf1": sh1, "shf2": sh2, "shf64": sh64, "shf32": sh32,
            "wmat": w, "iota": io, "iota2": io2, "iota3": io3}


def _pad_vinfo(v8):
    vpc = np.zeros((S, N, 2 * N), np.float32)
    vpc[:, :, :N] = v8
    vpcT = np.zeros((S, N, 2 * N), np.float32)
    vpcT[:, :, :N] = v8.transpose(0, 2, 1)
    # packed-phase arc score tables: partition p < 64 -> (i=p, s=sp);
    # p >= 64 -> (i=p-64, s=sp+4).  vl2[p,sp,c] = vinfo[s, i+c, i],
    # vr2[p,sp,c] = vinfo[s, i, i+c]  (garbage where i+c >= N).
    vl2 = np.zeros((N, 4, N), np.float32)
    vr2 = np.zeros((N, 4, N), np.float32)
    iidx = np.arange(64)
    for sp in range(4):
        for half, soff, poff in ((0, 0, 0), (1, 4, 64)):
            vv = v8[sp + soff]
            for c in range(N):
                jj = np.minimum(iidx + c, N - 1)
                vl2[poff + iidx, sp, c] = vv[jj, iidx]
                vr2[poff + iidx, sp, c] = vv[iidx, jj]
    # 2-lane tables: partition p -> group g=p//32, i=p%32, s=g*2+sp
    vl3 = np.zeros((N, 2, N), np.float32)
    vr3 = np.zeros((N, 2, N), np.float32)
    i3 = np.arange(32)
    for sp in range(2):
        for g in range(4):
            vv = v8[g * 2 + sp]
            for c in range(N):
                jj = np.minimum(i3 + c, N - 1)
                vl3[g * 32 + i3, sp, c] = vv[jj, i3]
                vr3[g * 32 + i3, sp, c] = vv[i3, jj]
    return vpc, vpcT, vl2, vr2, vl3, vr3


def _emit(tc, outs, ins):
    nc = tc.nc
    ctx = ExitStack()
    P = ctx.enter_context(tc.tile_pool(name="pers", bufs=1))
    SC = ctx.enter_context(tc.tile_pool(name="scr", bufs=4))
    S1 = ctx.enter_context(tc.tile_pool(name="scr1", bufs=3))
    SM = ctx.enter_context(tc.tile_pool(name="sml", bufs=6))
    PS = ctx.enter_context(tc.tile_pool(name="psum", bufs=2, space="PSUM"))

    # phase-1 tiles (8 sentences)
    A = P.tile([N, S, N], DT, tag="A")
    C = P.tile([N, S, N], DT, tag="C")
    E = P.tile([N, S, N], DT, tag="E")
    S00 = P.tile([N, S, N], DT, tag="S00")
    rev = {}
    for nm in ("B", "D", "F"):
        rev[nm] = [P.tile([N, S, N], DT, tag=f"{nm}{b}", name=f"{nm}{b}")
                   for b in range(3)]
    BT = {ab: P.tile([N, S, N], DI, tag=f"BT{ab}", name=f"BT{ab}")
          for ab in range(4)}
    vL = P.tile([N, S, N], DT, tag="vL")
    vR = P.tile([N, S, N], DT, tag="vR")
    # phase-2 tiles (4 sentence lanes, sentences 4-7 on partitions 64-127)
    A2 = P.tile([N, 4, N], DT, tag="A2")
    C2 = P.tile([N, 4, N], DT, tag="C2")
    E2 = P.tile([N, 4, N], DT, tag="E2")
    S00b = P.tile([N, 4, N], DT, tag="S00b")
    rev2 = {}
    for nm in ("B", "D", "F"):
        rev2[nm] = [P.tile([N, 4, N], DT, tag=f"{nm}{b}p", name=f"{nm}{b}p")
                    for b in range(3)]
    BT2 = {ab: P.tile([N, 4, N], DI, tag=f"BT{ab}p", name=f"BT{ab}p")
           for ab in range(4)}
    vL2 = P.tile([N, 4, N], DT, tag="vL2")
    vR2 = P.tile([N, 4, N], DT, tag="vR2")
    # phase-3 tiles (2 sentence lanes, 4 quadrant groups of i)
    A3 = P.tile([N, 2, N], DT, tag="A3")
    C3 = P.tile([N, 2, N], DT, tag="C3")
    E3 = P.tile([N, 2, N], DT, tag="E3")
    S00c = P.tile([N, 2, N], DT, tag="S00c")
    rev3 = {}
    for nm in ("B", "D", "F"):
        rev3[nm] = [P.tile([N, 2, N], DT, tag=f"{nm}{b}q", name=f"{nm}{b}q")
                    for b in range(3)]
    BT3 = {ab: P.tile([N, 2, N], DI, tag=f"BT{ab}q", name=f"BT{ab}q")
           for ab in range(4)}
    vL3 = P.tile([N, 2, N], DT, tag="vL3")
    vR3 = P.tile([N, 2, N], DT, tag="vR3")

    sh1 = P.tile([N, N], DT, tag="sh1")
    sh2 = P.tile([N, N], DT, tag="sh2")
    sh64 = P.tile([N, N], DT, tag="sh64")
    sh32 = P.tile([N, N], DT, tag="sh32")
    W = P.tile([N, N], DT, tag="W")
    Wh = P.tile([N, N], mybir.dt.bfloat16, tag="Wh")
    iof = P.tile([N, S], DT, tag="iof")
    iof2 = P.tile([N, 4], DT, tag="iof2")
    iof3 = P.tile([N, 2], DT, tag="iof3")
    zer = P.tile([N, S], DT, tag="zer")
    bonc = P.tile([N, 1], DT, tag="bonc")
    bigc = P.tile([N, 1], DT, tag="bigc")
    neg1c = P.tile([N, 1], DT, tag="neg1c")

    nc.gpsimd.dma_start(sh1[:, :], ins["shf1"])
    nc.gpsimd.dma_start(sh2[:, :], ins["shf2"])
    nc.gpsimd.dma_start(sh64[:, :], ins["shf64"])
    nc.gpsimd.dma_start(sh32[:, :], ins["shf32"])
    nc.gpsimd.dma_start(W[:, :], ins["wmat"])
    nc.vector.tensor_copy(Wh[:, :], W[:, :])
    nc.gpsimd.dma_start(iof[:, :], ins["iota"])
    nc.gpsimd.dma_start(iof2[:, :], ins["iota2"])
    nc.gpsimd.dma_start(iof3[:, :], ins["iota3"])
    nc.gpsimd.dma_start(vL2[:, :, :], ins["vl2"])
    nc.gpsimd.dma_start(vR2[:, :, :], ins["vr2"])
    nc.gpsimd.dma_start(vL3[:, :, :], ins["vl3"])
    nc.gpsimd.dma_start(vR3[:, :, :], ins["vr3"])
    # vL[i,s,k] = vinfo[s,i+k,i] = vpcT[s,i,i+k]; vR[i,s,k] = vpc[s,i,i+k]
    vhT = ins["vpcT"].tensor
    vh = ins["vpc"].tensor
    for s in range(S):
        nc.gpsimd.dma_start(
            vL[:, s, :], AP(vhT, s * 2 * N * N, [[2 * N + 1, N], [1, N]]))
        nc.gpsimd.dma_start(
            vR[:, s, :], AP(vh, s * 2 * N * N, [[2 * N + 1, N], [1, N]]))

    nc.vector.memset(zer[:, :], 0.0)
    nc.vector.memset(bonc[:, :], BON)
    nc.vector.memset(bigc[:, :], BIG)
    nc.vector.memset(neg1c[:, :], -1.0)
    for t in (A, C, E, S00):
        nc.vector.memset(t[:, :, :], NEGC)
        nc.gpsimd.memset(t[:, :, 0], 0.0)
    for nm in ("B", "D", "F"):
        for b in range(3):
            nc.gpsimd.memset(rev[nm][b][:, :, :], NEGC)
            nc.gpsimd.memset(rev2[nm][b][:, :, :], NEGC)
            nc.gpsimd.memset(rev3[nm][b][:, :, :], NEGC)
        nc.vector.memset(rev[nm][1][:, :, 1], 0.0)   # step1 slot1 = width0
        nc.vector.memset(rev[nm][2][:, :, 2], 0.0)   # step2 slot2 = width0
    for ab in range(4):
        nc.gpsimd.memset(BT[ab][:, :, :], 0)
        nc.gpsimd.memset(BT2[ab][:, :, :], 0)
        nc.gpsimd.memset(BT3[ab][:, :, :], 0)

    def step(k, Se, Ax, Cx, Ex, S00x, revx, BTx, vLx, vRx, iofx):
        Bk, Dk, Fk = (revx[nm][k % 3] for nm in ("B", "D", "F"))
        vLc = vLx[:, :, k]
        vRc = vRx[:, :, k]

        if k >= 2:
            c01 = SC.tile([N, Se, k - 1], DT, tag="c01")
            c11 = SC.tile([N, Se, k - 1], DT, tag="c11")
            if k >= 3:
                nc.gpsimd.tensor_tensor(
                    out=c01[:, :, 0:k - 2], in0=Cx[:, :, 1:k - 1],
                    in1=Dk[:, :, 1:k - 1], op=mybir.AluOpType.add)
                nc.gpsimd.tensor_tensor(
                    out=c11[:, :, 0:k - 2], in0=Ex[:, :, 1:k - 1],
                    in1=Fk[:, :, 1:k - 1], op=mybir.AluOpType.add)
            nc.gpsimd.tensor_tensor(
                out=c01[:, :, k - 2:k - 1], in0=Cx[:, :, k - 1:k],
                in1=Dk[:, :, k - 1:k], op=mybir.AluOpType.add)
            nc.gpsimd.tensor_tensor(
                out=c11[:, :, k - 2:k - 1], in0=Ex[:, :, k - 1:k],
                in1=Fk[:, :, k - 1:k], op=mybir.AluOpType.add)
        base = SC.tile([N, Se, k], DT, tag="base")
        nc.vector.tensor_tensor(
            out=base[:, :, :], in0=Ax[:, :, 0:k],
            in1=Bk[:, :, 1:k + 1], op=mybir.AluOpType.add)
        rb = SM.tile([N, Se], DT, tag="rb")
        nc.vector.tensor_reduce(rb[:, :], base[:, :, :],
                                axis=mybir.AxisListType.X, op=mybir.AluOpType.max)
        if k >= 2:
            m01i = SM.tile([N, Se], DT, tag="m01i")
            nc.vector.tensor_reduce(m01i[:, :], c01[:, :, :],
                                    axis=mybir.AxisListType.X,
                                    op=mybir.AluOpType.max)
            m11i = SM.tile([N, Se], DT, tag="m11i")
            nc.vector.tensor_reduce(m11i[:, :], c11[:, :, :],
                                    axis=mybir.AxisListType.X,
                                    op=mybir.AluOpType.max)

        # values (exact reference fp order)
        t0 = SM.tile([N, Se], DT, tag="t0")
        nc.vector.tensor_tensor(out=t0[:, :], in0=rb[:, :], in1=vLc,
                                op=mybir.AluOpType.add)
        nc.scalar.add(Dk[:, :, 0], t0[:, :], bonc[:, :])                      # m00
        t1 = SM.tile([N, Se], DT, tag="t1")
        nc.vector.tensor_tensor(out=t1[:, :], in0=rb[:, :], in1=vRc,
                                op=mybir.AluOpType.add)
        nc.scalar.add(Ex[:, :, k], t1[:, :], bonc[:, :])                      # m10
        nc.scalar.copy(S00x[:, :, k], Dk[:, :, 0])

        t2 = SM.tile([N, Se], DT, tag="t2")
        nc.vector.tensor_tensor(out=t2[:, :], in0=base[:, :, 0], in1=vLc,
                                op=mybir.AluOpType.add)
        part00 = SM.tile([N, Se], DT, tag="part00")
        nc.scalar.add(part00[:, :], t2[:, :], bonc[:, :])

        if k >= 2:
            nc.vector.tensor_tensor(out=Bk[:, :, 0], in0=part00[:, :],
                                    in1=m01i[:, :], op=mybir.AluOpType.max)
            nc.vector.tensor_tensor(out=Fk[:, :, 0], in0=m11i[:, :],
                                    in1=Ex[:, :, k], op=mybir.AluOpType.max)
        else:
            nc.vector.tensor_copy(Bk[:, :, 0], part00[:, :])
            nc.vector.tensor_copy(Fk[:, :, 0], Ex[:, :, k])
        nc.scalar.copy(Cx[:, :, k], Bk[:, :, 0])
        nc.scalar.copy(Ax[:, :, k], Fk[:, :, 0])

        # PE slides
        if k <= N - 2:
            Bn, Dn, Fn = (revx[nm][(k + 1) % 3] for nm in ("B", "D", "F"))
            psF = PS.tile([N, 3 * Se], DT, tag="psF")
            for ci, (cur, nxt) in enumerate(((Bk, Bn), (Dk, Dn), (Fk, Fn))):
                nc.tensor.matmul(psF[:, ci * Se:(ci + 1) * Se], sh1[:, :],
                                 cur[:, :, 0], start=True, stop=True)
                nc.scalar.copy(nxt[:, :, 1], psF[:, ci * Se:(ci + 1) * Se])
        if k <= N - 3:
            B2r, D2r, F2r = (revx[nm][(k + 2) % 3] for nm in ("B", "D", "F"))
            wlen = k + 1                     # slots [0..k]
            wmax = 512 // Se
            w1 = min(wmax, wlen)
            for cur, nxt in ((Bk, B2r), (Dk, D2r), (Fk, F2r)):
                psB = PS.tile([N, 2, 512], DT, tag="psB")
                nc.tensor.matmul(psB[:, 0, 0:Se * w1], sh2[:, :],
                                 cur[:, :, 0:w1], start=True, stop=True)
                nc.scalar.copy(
                    nxt[:, :, 2:w1 + 2],
                    psB[:, 0, 0:Se * w1].rearrange("p (s w) -> p s w", w=w1))
                if wlen > wmax:
                    w2 = wlen - wmax
                    nc.tensor.matmul(psB[:, 1, 0:Se * w2], sh2[:, :],
                                     cur[:, :, wmax:wlen], start=True, stop=True)
                    nc.scalar.copy(
                        nxt[:, :, wmax + 2:wlen + 2],
                        psB[:, 1, 0:Se * w2].rearrange("p (s w) -> p s w", w=w2))

        # argmax 00/10 (shared)
        eqb = S1.tile([N, Se, k], mybir.dt.bfloat16, tag="eqb")
        nc.vector.tensor_tensor(out=eqb[:, :, :], in0=base[:, :, :],
                                in1=rb[:, :].unsqueeze(2).broadcast_to([N, Se, k]),
                                op=mybir.AluOpType.is_ge)
        tb = S1.tile([N, Se, k], mybir.dt.bfloat16, tag="tb")
        nc.gpsimd.tensor_tensor(out=tb[:, :, :], in0=eqb[:, :, :],
                                in1=Wh[:, 0:k].unsqueeze(1).broadcast_to([N, Se, k]),
                                op=mybir.AluOpType.mult)
        amb = SM.tile([N, Se], mybir.dt.bfloat16, tag="amb")
        nc.vector.tensor_reduce(amb[:, :], tb[:, :, :],
                                axis=mybir.AxisListType.X, op=mybir.AluOpType.max)
        ms0 = SM.tile([N, Se], DT, tag="ms0")
        nc.scalar.activation(ms0[:, :], amb[:, :],
                             mybir.ActivationFunctionType.Identity,
                             bias=bigc[:, :], scale=neg1c[:, :])
        nc.vector.tensor_tensor(out=BTx[0][:, :, k], in0=ms0[:, :], in1=iofx,
                                op=mybir.AluOpType.add)
        nc.gpsimd.tensor_copy(BTx[2][:, :, k], BTx[0][:, :, k])

        # argmax 01
        if k >= 2:
            eq1 = S1.tile([N, Se, k - 1], mybir.dt.bfloat16, tag="eq1")
            nc.vector.tensor_tensor(
                out=eq1[:, :, :], in0=c01[:, :, :],
                in1=m01i[:, :].unsqueeze(2).broadcast_to([N, Se, k - 1]),
                op=mybir.AluOpType.is_ge)
            t1m = S1.tile([N, Se, k - 1], mybir.dt.bfloat16, tag="t1m")
            nc.gpsimd.tensor_tensor(
                out=t1m[:, :, :], in0=eq1[:, :, :],
                in1=Wh[:, 1:k].unsqueeze(1).broadcast_to([N, Se, k - 1]),
                op=mybir.AluOpType.mult)
            am1 = SM.tile([N, Se], mybir.dt.bfloat16, tag="am1")
            nc.vector.tensor_reduce(am1[:, :], t1m[:, :, :],
                                    axis=mybir.AxisListType.X,
                                    op=mybir.AluOpType.max)
            ms1 = SM.tile([N, Se], DT, tag="ms1")
            nc.scalar.activation(ms1[:, :], am1[:, :],
                                 mybir.ActivationFunctionType.Identity,
                                 bias=bigc[:, :], scale=neg1c[:, :])
            ge1 = SM.tile([N, Se], DI, tag="ge1")
            nc.vector.tensor_tensor(out=ge1[:, :], in0=part00[:, :],
                                    in1=m01i[:, :], op=mybir.AluOpType.is_ge)
            nc.vector.copy_predicated(ms1[:, :], ge1[:, :], zer[:, 0:Se])
            nc.vector.tensor_tensor(out=BTx[1][:, :, k], in0=ms1[:, :],
                                    in1=iofx, op=mybir.AluOpType.add)
        else:
            nc.vector.tensor_copy(BTx[1][:, :, k], iofx)

        # argmax 11
        if k >= 2:
            eq2 = S1.tile([N, Se, k - 1], mybir.dt.bfloat16, tag="eq2")
            nc.vector.tensor_tensor(
                out=eq2[:, :, :], in0=c11[:, :, :],
                in1=m11i[:, :].unsqueeze(2).broadcast_to([N, Se, k - 1]),
                op=mybir.AluOpType.is_ge)
            t2m = S1.tile([N, Se, k - 1], mybir.dt.bfloat16, tag="t2m")
            nc.gpsimd.tensor_tensor(
                out=t2m[:, :, :], in0=eq2[:, :, :],
                in1=Wh[:, 1:k].unsqueeze(1).broadcast_to([N, Se, k - 1]),
                op=mybir.AluOpType.mult)
            am2 = SM.tile([N, Se], mybir.dt.bfloat16, tag="am2")
            nc.vector.tensor_reduce(am2[:, :], t2m[:, :, :],
                                    axis=mybir.AxisListType.X,
                                    op=mybir.AluOpType.max)
            ms2 = SM.tile([N, Se], DT, tag="ms2")
            nc.scalar.activation(ms2[:, :], am2[:, :],
                                 mybir.ActivationFunctionType.Identity,
                                 bias=bigc[:, :], scale=neg1c[:, :])
            q11 = SM.tile([N, Se], DT, tag="q11")
            nc.vector.tensor_tensor(out=q11[:, :], in0=ms2[:, :], in1=iofx,
                                    op=mybir.AluOpType.add)
            ge2 = SM.tile([N, Se], DI, tag="ge2")
            nc.vector.tensor_tensor(out=ge2[:, :], in0=m11i[:, :],
                                    in1=Ex[:, :, k], op=mybir.AluOpType.is_ge)
            jkt = SM.tile([N, Se], DT, tag="jk")
            nc.vector.tensor_scalar_add(jkt[:, :], iofx, float(k))
            nc.vector.copy_predicated(jkt[:, :], ge2[:, :], q11[:, :])
            nc.vector.tensor_copy(BTx[3][:, :, k], jkt[:, :])
        else:
            jkt = SM.tile([N, Se], DT, tag="jk")
            nc.vector.tensor_scalar_add(jkt[:, :], iofx, float(k))
            nc.vector.tensor_copy(BTx[3][:, :, k], jkt[:, :])

    # ---- phase 1: k = 1..KSPLIT-1, 8 sentences on free axis ----
    for k in range(1, KSPLIT):
        step(k, S, A, C, E, S00, rev, BT, vL, vR, iof[:, :])

    # ---- transition: repack sentences 4-7 onto partitions 64-127 ----
    # charts A/C/E -> A2/C2/E2 (all 128 cols; only i<64 rows matter)
    def repack(dst, src, cols):
        # dst[0:64, :, 0:cols] = src[0:64, 0:4, 0:cols]
        nc.scalar.copy(dst[0:64, :, 0:cols], src[0:64, 0:4, 0:cols])
        # dst[64:128] = P64 @ src[:, 4:8]
        nchk = (4 * cols + 511) // 512
        psR = PS.tile([N, 2, 512], DT, tag="psB")
        cw = (cols + nchk - 1) // nchk
        for ci in range(nchk):
            c0 = ci * cw
            c1 = min(cols, c0 + cw)
            nc.tensor.matmul(psR[:, ci % 2, 0:4 * (c1 - c0)], sh64[:, :],
                             src[:, 4:8, c0:c1], start=True, stop=True)
            nc.scalar.copy(
                dst[64:128, :, c0:c1],
                psR[64:128, ci % 2, 0:4 * (c1 - c0)].rearrange(
                    "p (s w) -> p s w", s=4))

    repack(A2, A, KSPLIT)
    repack(C2, C, KSPLIT)
    repack(E2, E, KSPLIT)
    g1, g2 = KSPLIT % 3, (KSPLIT + 1) % 3
    for nm in ("B", "D", "F"):
        repack(rev2[nm][g1], rev[nm][g1], KSPLIT + 2)
        repack(rev2[nm][g2], rev[nm][g2], KSPLIT + 3)

    # ---- phase 2: k = KSPLIT..KSPLIT2-1, 4 sentence lanes ----
    for k in range(KSPLIT, KSPLIT2):
        step(k, 4, A2, C2, E2, S00b, rev2, BT2, vL2, vR2, iof2[:, :])

    # ---- transition 2: lanes 2-3 shift +32 partitions (both halves) ----
    def repack3(dst, src_t, cols):
        # lanes 0-1 stay put (groups 0 and 2 already in place)
        nc.scalar.copy(dst[:, :, 0:cols], src_t[:, 0:2, 0:cols])
        nchk = (2 * cols + 511) // 512
        psR = PS.tile([N, 2, 512], DT, tag="psB")
        cw = (cols + nchk - 1) // nchk
        for ci in range(nchk):
            c0 = ci * cw
            c1 = min(cols, c0 + cw)
            nc.tensor.matmul(psR[:, ci % 2, 0:2 * (c1 - c0)], sh32[:, :],
                             src_t[:, 2:4, c0:c1], start=True, stop=True)
            for p0 in (32, 96):
                nc.scalar.copy(
                    dst[p0:p0 + 32, :, c0:c1],
                    psR[p0:p0 + 32, ci % 2, 0:2 * (c1 - c0)].rearrange(
                        "p (s w) -> p s w", s=2))

    if KSPLIT2 < N:
        repack3(A3, A2, N)
        repack3(C3, C2, N)
        repack3(E3, E2, N)
        h1, h2 = KSPLIT2 % 3, (KSPLIT2 + 1) % 3
        for nm in ("B", "D", "F"):
            repack3(rev3[nm][h1], rev2[nm][h1], min(KSPLIT2 + 2, N))
            repack3(rev3[nm][h2], rev2[nm][h2], min(KSPLIT2 + 3, N))

    # ---- phase 3: k = KSPLIT2..127, 2 sentence lanes ----
    for k in range(KSPLIT2, N):
        step(k, 2, A3, C3, E3, S00c, rev3, BT3, vL3, vR3, iof3[:, :])

    # deskew: dram flat idx (per sentence) = i*257 + w  (= i*256 + j, j=i+w)
    KS = KSPLIT
    K2 = KSPLIT2
    for s in range(S):
        poff = 0 if s < 4 else 64
        sp = s % 4
        qoff = (s // 2) * 32
        qp = s % 2
        for eng, nm, old_t, new_t, q_t in (
                ("sync", "sc00", S00, S00b, S00c), ("sync", "sc01", C, C2, C3),
                ("sync", "sc10", E, E2, E3), ("sync", "sc11", A, A2, A3),
                ("gp", "bt00", BT[0], BT2[0], BT3[0]),
                ("gp", "bt01", BT[1], BT2[1], BT3[1]),
                ("gp", "bt10", BT[2], BT2[2], BT3[2]),
                ("gp", "bt11", BT[3], BT2[3], BT3[3])):
            h = outs[nm].tensor
            e = nc.sync if eng == "sync" else nc.gpsimd
            e.dma_start(AP(h, s * N * 256, [[257, N], [1, KS]]),
                        old_t[:, s, 0:KS])
            e.dma_start(AP(h, s * N * 256 + KS, [[257, 64], [1, K2 - KS]]),
                        new_t[poff:poff + 64, sp, KS:K2])
            if N - K2 > 0:
                e.dma_start(AP(h, s * N * 256 + K2, [[257, 32], [1, N - K2]]),
                            q_t[qoff:qoff + 32, qp, K2:N])
    ctx.close()


_NC_CACHE = None


def _build():
    global _NC_CACHE
    if _NC_CACHE is not None:
        return _NC_CACHE
    nc = bacc.Bacc("TRN2", target_bir_lowering=False, debug=False,
                   enable_asserts=False, num_devices=NCORES)
    ins = {nm: nc.dram_tensor(nm, sh, DT, kind="ExternalInput").ap()
           for nm, sh in IN_SPECS.items()}
    outs = {}
    for nm in OUT_NAMES:
        dt = DT if nm.startswith("sc") else DI
        outs[nm] = nc.dram_tensor(nm, [S, N, 2 * N], dt,
                                  kind="ExternalOutput").ap()
    with tile.TileContext(nc) as tc:
        _emit(tc, outs, ins)
    nc.compile()
    _NC_CACHE = nc
    return nc


_LAST_EXEC_NS = None


def kernel(b_vinfo_mtx, b_buffer_size, _trace=False):
    global _LAST_EXEC_NS
    v = np.ascontiguousarray(np.asarray(b_vinfo_mtx, dtype=np.float32))
    assert v.shape == (NCORES * S, N, N)
    consts = _host_consts()
    in_maps = []
    for c in range(NCORES):
        vpc, vpcT, vl2, vr2, vl3, vr3 = _pad_vinfo(v[c * S:(c + 1) * S])
        in_maps.append({"vpc": vpc, "vpcT": vpcT, "vl2": vl2, "vr2": vr2,
                        "vl3": vl3, "vr3": vr3, **consts})

    nc = _build()
    res = bass_utils.run_bass_kernel_spmd(
        nc, in_maps, core_ids=list(range(NCORES)), trace=_trace)
    _LAST_EXEC_NS = res.exec_time_ns

    scores = np.full((NCORES * S, N, N, 2, 2), NEGC, np.float32)
    bt = np.zeros((NCORES * S, N, N, 2, 2), np.int32)
    names = {"sc00": (0, 0), "sc01": (0, 1), "sc10": (1, 0), "sc11": (1, 1)}
    tri = np.tril_indices(N, k=-1)
    for c in range(NCORES):
        r = res.results[c]
        for nm, (a, b) in names.items():
            sc = r[nm].reshape(S, N, 2 * N)[:, :, :N].copy()
            bb = r["bt" + nm[2:]].reshape(S, N, 2 * N)[:, :, :N].copy()
            sc[:, tri[0], tri[1]] = NEGC
            bb[:, tri[0], tri[1]] = 0
            # invalid upper cells (j >= N would be needed) are never written
            # in phase 2 for i >= 64... host masks the lower triangle only;
            # upper-triangle invalid cells beyond buffer are not part of the
            # reference output range.
            scores[c * S:(c + 1) * S, :, :, a, b] = sc
            bt[c * S:(c + 1) * S, :, :, a, b] = bb
    return scores, bt
